# revision 43
# baseline (speedup 1.0000x reference)
"""BERT self-attention (BS=4, SEQ=2048, HID=768, NH=12) on 8 NeuronCores.

Sharding: core c -> batch b = c//2, head-group g = c%2 (6 heads each).

v8 design (573us baseline -> 337 -> 295 -> 292 -> 285 -> this):
  * Softmax denominator comes free from the ctx matmul: V is stored per
    head as 65 columns (64 V dims + the 0/1 mask column), so ctx PSUM
    row 64 accumulates sum_k m_k * P[k,q].  No denominator matmuls.
  * Scores for the head pair are packed side by side in one
    [128k, 2*512q] PSUM tile -> ONE exp per kb iteration; the two
    64-dim score matmuls run concurrently in PE row halves.
  * The PE (~0.5 ns/column streaming) is the global bottleneck, so all
    projection work is interleaved as filler and every spare matmul
    column is trimmed (V bias is applied at the drain on DVE instead of
    16 ones-row matmuls; the broadcast ones vector is memset on chip).
  * Host pre-packs X^T and W^T into partition-major [128, 6*N] layouts:
    input DMAs are fully contiguous (max descriptor efficiency), split
    across the sync (X) and scalar (weights) DGE queues.  Mask and
    biases ride in spare columns of the wk param.
  * The (j,qc) drain is software-pipelined across the sweep boundary
    (reciprocal work at iter 0, broadcast matmul + multiply + output
    DMA at iter 1) and the ctx accumulators are copied out of PSUM
    immediately so the banks recycle; the final drain runs its copies
    on the then-idle ACT engine.
  * 1/denom via DVE reciprocal_approx_fast on an SBUF-staged row (the
    custom op drops partition offsets).  GPSIMD compute is avoided:
    touching it downclocks the whole core ~20%.

PSUM (8 banks): scores 2x[128,1024] (4) + ctx 2x[65,512] (2) +
projection/V/broadcast 2x[128,512] (2); the prologue borrows 3 banks
before the attention pools open.
"""

import numpy as np

import concourse.bass as bass
import concourse.tile as tile
from concourse import bacc
from concourse import mybir
from concourse.bass_utils import run_bass_kernel_spmd

F32 = mybir.dt.float32
F16 = mybir.dt.float16
DT_MM = F16
DT_NP = np.float16

BS, SEQ, HID, NH, HD = 4, 2048, 768, 12, 64
NCORES = 8
HPC = 6          # heads per core
FCH = 6          # 128-row chunks of the 768 contraction dim
DSH = HPC * HD   # 384 output features per core
QC = 4           # q chunks of 512
KB = 16          # k blocks of 128
WKX = FCH * DSH  # start of the mask/bias columns in the wk param

FILL = {
    (0, 0): {2: ("k", 0, 1), 6: ("k", 0, 2), 10: ("k", 0, 3),
             14: ("q", 0, 1)},
    (0, 1): {6: ("q", 0, 2), 10: ("k", 1, 0), 13: ("k", 1, 1)},
    (0, 2): {6: ("q", 0, 3), 10: ("k", 1, 2), 13: ("k", 1, 3)},
    (0, 3): {8: ("q", 1, 0)},
    (1, 0): {6: ("q", 1, 1), 10: ("k", 2, 0), 13: ("k", 2, 1)},
    (1, 1): {6: ("q", 1, 2), 10: ("k", 2, 2), 13: ("k", 2, 3)},
    (1, 2): {8: ("q", 1, 3)},
    (1, 3): {8: ("q", 2, 0)},
    (2, 0): {8: ("q", 2, 1)},
    (2, 1): {8: ("q", 2, 2)},
    (2, 2): {8: ("q", 2, 3)},
    (2, 3): {},
}


def _body(tc, xt_d, wq_d, wk_d, wv_d, ot_d):
    nc = tc.nc
    Exp = mybir.ActivationFunctionType.Exp

    with tc.tile_pool(name="persist", bufs=1) as persist:
        # Warm the exp table ASAP (overlaps the input DMAs).
        dummy = persist.tile([1, 1], F32, tag="dummy")
        nc.vector.memset(dummy, 0.0)
        nc.scalar.activation(out=dummy, in_=dummy, func=Exp)

        # Contiguous partition-major input DMAs on two parallel DGE
        # queues: X (split for chunk-level pipelining) on sync, weights
        # on scalar.  GPSIMD's queue is untouched: ANY GpSimd engagement
        # (even a DMA trigger) downclocks the whole core ~20%.
        # X arrives in 4 position-blocks (columns 512c..512c+511): the
        # first scores need only block 0, later blocks feed the V/K JIT.
        # Two parallel DGE queues: X on sync, weights on scalar (the
        # gpsimd queue starts too slowly to help).
        wkp = persist.tile([128, WKX + 28], DT_MM, tag="wkp")
        nc.scalar.dma_start(out=wkp, in_=wk_d[:, :])
        xtp = persist.tile([128, QC, FCH, 512], DT_MM, tag="xtp")
        for c in range(3):
            nc.sync.dma_start(out=xtp[:, c, :, :], in_=xt_d[:, c, :, :])
        wqp = persist.tile([128, FCH, DSH], DT_MM, tag="wqp")
        nc.scalar.dma_start(out=wqp, in_=wq_d[:, :, :])
        wvp = persist.tile([128, FCH, DSH], DT_MM, tag="wvp")
        nc.scalar.dma_start(out=wvp, in_=wv_d[:, :, :])
        # scalar queue idles after the weights while sync still streams
        # X: it takes the last X block (needed latest, at kb=12).
        nc.scalar.dma_start(out=xtp[:, 3, :, :], in_=xt_d[:, 3, :, :])

        # Mask / bias views and casts (wkp cols WKX..): 16 mask, 3 bq,
        # 3 bk, 6 bv (64 rows each).
        mtile = wkp[:, WKX:WKX + KB]
        mtf = persist.tile([128, KB], F32, tag="mtf")
        nc.vector.tensor_copy(out=mtf, in_=mtile)
        qkb = persist.tile([128, 6], F32, tag="qkb")
        nc.vector.tensor_copy(out=qkb, in_=wkp[:, WKX + KB:WKX + KB + 6])
        bvt = persist.tile([128, 6], F32, tag="bvt")
        nc.vector.tensor_copy(out=bvt, in_=wkp[:, WKX + KB + 6:WKX + 28])
        ones64 = persist.tile([1, HD], DT_MM, tag="ones64")
        nc.vector.memset(ones64, 1.0)

        def xq(qc, f):
            # X^T rows f*128.., columns qc*512..(qc+1)*512
            return xtp[:, qc, f, :]

        def xk(kb, f):
            # X^T rows f*128.., columns kb*128..(kb+1)*128
            return xtp[:, kb // 4, f, (kb % 4) * 128:(kb % 4 + 1) * 128]

        qt = [persist.tile([128, SEQ], DT_MM, tag=f"qt{j}", name=f"qt{j}")
              for j in range(3)]
        kt = [persist.tile([128, SEQ], DT_MM, tag=f"kt{j}", name=f"kt{j}")
              for j in range(3)]
        # V with per-head mask column: [k=128, kb, head, 64 V dims + m].
        vt = persist.tile([128, KB, HPC, HD + 1], DT_MM, tag="vt")
        for h in range(HPC):
            nc.vector.tensor_copy(out=vt[:, :, h, HD], in_=mtile)
        ostage = [persist.tile([64, SEQ], F32, tag=f"os{h}", name=f"os{h}")
                  for h in range(HPC)]

        def make_proj(fpool):
            def proj_chunk(kind, j, qc):
                """Q or K projection chunk -> qt/kt[j][:, qc*512:...],
                bias folded into the DVE drain."""
                ps = fpool.tile([128, 512], F32, tag="f", name="fq")
                qs = slice(qc * 512, (qc + 1) * 512)
                wt = wqp if kind == "q" else wkp
                for f in range(FCH):
                    nc.tensor.matmul(
                        ps,
                        lhsT=wt[:, f * DSH + j * 128:f * DSH + (j + 1) * 128]
                        if kind == "k" else wt[:, f, j * 128:(j + 1) * 128],
                        rhs=xq(qc, f),
                        start=(f == 0), stop=(f == FCH - 1))
                dst = (qt if kind == "q" else kt)[j]
                bcol = (0 if kind == "q" else 3) + j
                nc.vector.tensor_scalar_add(out=dst[:, qs], in0=ps,
                                            scalar1=qkb[:, bcol:bcol + 1])

            def v_chunk(kb):
                """V k-block kb -> vt[:, kb, :, 0:64], mask-scaled rows.
                bv is applied at the drain (out = ctx/denom + bv)."""
                ps = fpool.tile([128, DSH], F32, tag="f", name="fv")
                for f in range(FCH):
                    nc.tensor.matmul(ps, lhsT=xk(kb, f),
                                     rhs=wvp[:, f, :],
                                     start=(f == 0), stop=(f == FCH - 1))
                nc.vector.tensor_scalar_mul(
                    out=vt[:, kb, :, 0:HD], in0=ps,
                    scalar1=mtf[:, kb:kb + 1])

            return proj_chunk, v_chunk

        # Prologue (overlaps the input DMA stream) in its own multi-buffer
        # PSUM pool so chunks pipeline at PE speed.
        with tc.tile_pool(name="pre", bufs=3, space="PSUM") as pre:
            proj_chunk, v_chunk = make_proj(pre)
            proj_chunk("k", 0, 0)
            proj_chunk("q", 0, 0)
            v_chunk(0)
            v_chunk(1)

        with tc.tile_pool(name="sp", bufs=2, space="PSUM") as sp, \
             tc.tile_pool(name="cp", bufs=2, space="PSUM") as cp, \
             tc.tile_pool(name="fp", bufs=2, space="PSUM") as fp, \
             tc.tile_pool(name="pp", bufs=6) as pp, \
             tc.tile_pool(name="rdp", bufs=2) as rdp:
            proj_chunk, v_chunk = make_proj(fp)

            def drain_p1(st, on_act=False):
                """First drain stage: copy ctx out of PSUM (frees the
                accumulator banks) and build the f16 reciprocal rows."""
                cpy = nc.scalar.copy if on_act else (
                    lambda out, in_: nc.vector.tensor_copy(out=out, in_=in_))
                st["cs"], st["rd"] = [], []
                for i in range(2):
                    cs = rdp.tile([64, 512], F32, tag=f"cs{i}", name="cs")
                    cpy(out=cs, in_=st["ctx"][i][0:HD, :])
                    dn = rdp.tile([1, 512], F32, tag="dn", name="dn")
                    cpy(out=dn, in_=st["ctx"][i][HD:HD + 1, :])
                    r32 = rdp.tile([1, 512], F32, tag="r32", name="r32")
                    nc.vector.reciprocal_approx_fast(out=r32, in_=dn)
                    rd = rdp.tile([1, 512], DT_MM, tag="r16", name="rd")
                    nc.vector.tensor_copy(out=rd, in_=r32)
                    st["cs"].append(cs)
                    st["rd"].append(rd)

            def drain_p2(st, on_act=False):
                """Second drain stage: broadcast 1/denom (ones matmul),
                multiply, add bv, stream the output DMA."""
                cpy = nc.scalar.copy if on_act else (
                    lambda out, in_: nc.vector.tensor_copy(out=out, in_=in_))
                for i in range(2):
                    h = st["heads"][i]
                    bc = fp.tile([64, 512], F32, tag="f", name="bc")
                    nc.tensor.matmul(bc, lhsT=ones64, rhs=st["rd"][i],
                                     start=True, stop=True)
                    bcs = rdp.tile([64, 512], F32, tag="bcs", name="bcs")
                    cpy(out=bcs, in_=bc)
                    mo = rdp.tile([64, 512], F32, tag="mo", name="mo")
                    nc.vector.tensor_mul(out=mo, in0=st["cs"][i], in1=bcs)
                    nc.vector.tensor_scalar_add(
                        out=ostage[h][:, st["qs"]], in0=mo,
                        scalar1=bvt[0:HD, h:h + 1])
                    nc.sync.dma_start(out=ot_d[h][:, st["qs"]],
                                      in_=ostage[h][:, st["qs"]])

            # Uniform software pipeline over all 192 (j,qc,kb) iterations:
            # at iteration t the PE stream is S(t), C(t-1), filler —
            # including across sweep boundaries.
            prevc = None
            pending = None
            for j in range(3):
                heads = (2 * j, 2 * j + 1)
                for qc in range(QC):
                    qs = slice(qc * 512, (qc + 1) * 512)
                    fill_at = FILL[(j, qc)]
                    ctx = [cp.tile([HD + 1, 512], F32, tag="c", name=f"ctx{i}")
                           for i in range(2)]
                    for kb in range(KB):
                        ks = slice(kb * 128, (kb + 1) * 128)
                        sab = sp.tile([128, 1024], F32, tag="s", name="sab")
                        # Scores gate the ACT stream (the critical engine):
                        # highest priority so ready fillers never preempt.
                        with tc.high_priority():
                            for i in range(2):
                                rows = slice(64 * i, 64 * (i + 1))
                                nc.tensor.matmul(sab[:, 512 * i:512 * (i + 1)],
                                                 lhsT=kt[j][rows, ks],
                                                 rhs=qt[j][rows, qs],
                                                 start=True, stop=True,
                                                 skip_group_check=True)
                        if prevc is not None:
                            pctx, pheads, pkb, pp_ = prevc
                            for i in range(2):
                                nc.tensor.matmul(
                                    pctx[i],
                                    lhsT=vt[:, pkb, pheads[i], :],
                                    rhs=pp_[:, 512 * i:512 * (i + 1)],
                                    start=(pkb == 0), stop=(pkb == KB - 1))
                        if kb == 0 and pending is not None:
                            drain_p1(pending)
                        if kb == 1 and pending is not None:
                            drain_p2(pending)
                            pending = None
                        if j == 0 and qc == 0 and kb < KB - 2:
                            v_chunk(kb + 2)
                        if kb in fill_at:
                            proj_chunk(*fill_at[kb])
                        p = pp.tile([128, 1024], DT_MM, tag="p", name="ptile")
                        nc.scalar.activation(out=p, in_=sab, func=Exp,
                                             scale=0.125)
                        prevc = (ctx, heads, kb, p)
                    pending = {"ctx": ctx, "heads": heads, "qs": qs}

            # Tail: final ctx pair, then the last drain with its copies on
            # the now-idle ACT engine.
            pctx, pheads, pkb, pp_ = prevc
            for i in range(2):
                nc.tensor.matmul(pctx[i], lhsT=vt[:, pkb, pheads[i], :],
                                 rhs=pp_[:, 512 * i:512 * (i + 1)],
                                 start=False, stop=True)
            drain_p1(pending, on_act=True)
            drain_p2(pending, on_act=True)


def build_nc():
    nc = bacc.Bacc("TRN2")
    xt_d = nc.declare_dram_parameter("xtp", [128, QC, FCH, 512], DT_MM, isOutput=False)
    wq_d = nc.declare_dram_parameter("wqp", [128, FCH, DSH], DT_MM, isOutput=False)
    wk_d = nc.declare_dram_parameter("wkp", [128, WKX + 28], DT_MM, isOutput=False)
    wv_d = nc.declare_dram_parameter("wvp", [128, FCH, DSH], DT_MM, isOutput=False)
    ot_d = nc.declare_dram_parameter("OT", [HPC, HD, SEQ], F32, isOutput=True)
    with tile.TileContext(nc) as tc:
        _body(tc, xt_d, wq_d, wk_d, wv_d, ot_d)
    nc.finalize()
    return nc


_NC_CACHE = None


def _get_nc():
    global _NC_CACHE
    if _NC_CACHE is None:
        _NC_CACHE = build_nc()
    return _NC_CACHE


def _pack_pm(m):
    """[768, N] -> partition-major [128, 6*N] (chunk-major free dim)."""
    n = m.shape[1]
    return np.ascontiguousarray(
        m.reshape(FCH, 128, n).transpose(1, 0, 2).reshape(128, FCH * n))


def make_in_maps(hidden_states, attention_mask, Wq, bq, Wk, bk, Wv, bv):
    in_maps = []
    for c in range(NCORES):
        b, g = c // 2, c % 2
        hs = slice(g * DSH, (g + 1) * DSH)
        # [128, qc-block, f-chunk, 512]: xtp[p,c,f,q] = X^T[f*128+p, c*512+q]
        xtp = np.ascontiguousarray(
            hidden_states[b].T.astype(DT_NP)
            .reshape(FCH, 128, QC, 512).transpose(1, 2, 0, 3))
        wqp = _pack_pm(Wq[hs, :].T.astype(DT_NP))
        wvp = _pack_pm(Wv[hs, :].T.astype(DT_NP))

        wkp = np.zeros((128, WKX + 28), DT_NP)
        wkp[:, :WKX] = _pack_pm(Wk[hs, :].T.astype(DT_NP))
        m = (attention_mask[b, 0, 0] > -1).astype(DT_NP)
        wkp[:, WKX:WKX + KB] = m.reshape(KB, 128).T
        for j in range(3):
            wkp[:, WKX + KB + j] = bq[g * DSH + j * 128: g * DSH + (j + 1) * 128]
            wkp[:, WKX + KB + 3 + j] = bk[g * DSH + j * 128: g * DSH + (j + 1) * 128]
        for h in range(HPC):
            wkp[0:HD, WKX + KB + 6 + h] = bv[g * DSH + h * HD: g * DSH + (h + 1) * HD]

        in_maps.append({"xtp": xtp, "wqp": wqp, "wkp": wkp, "wvp": wvp})
    return in_maps


def gather_out(results):
    out = np.empty((BS, SEQ, HID), np.float32)
    for c in range(NCORES):
        b, g = c // 2, c % 2
        ot = results[c]["OT"]  # [6, 64, 2048]
        out[b, :, g * DSH:(g + 1) * DSH] = (
            ot.transpose(2, 0, 1).reshape(SEQ, DSH)
        )
    return out


def kernel(hidden_states, attention_mask, Wq, bq, Wk, bk, Wv, bv):
    nc = _get_nc()
    in_maps = make_in_maps(hidden_states, attention_mask,
                           Wq, bq, Wk, bk, Wv, bv)
    res = run_bass_kernel_spmd(nc, in_maps, core_ids=list(range(NCORES)))
    return gather_out(res.results)


# revision 44
# speedup vs baseline: 1.1881x; 1.1881x over previous
"""BERT self-attention (BS=4, SEQ=2048, HID=768, NH=12) on 8 NeuronCores.

Sharding: core c -> batch b = c//2, head-group g = c%2 (6 heads each).

v8 design (573us baseline -> 337 -> 295 -> 292 -> 285 -> this):
  * Softmax denominator comes free from the ctx matmul: V is stored per
    head as 65 columns (64 V dims + the 0/1 mask column), so ctx PSUM
    row 64 accumulates sum_k m_k * P[k,q].  No denominator matmuls.
  * Scores for the head pair are packed side by side in one
    [128k, 2*512q] PSUM tile -> ONE exp per kb iteration; the two
    64-dim score matmuls run concurrently in PE row halves.
  * The PE (~0.5 ns/column streaming) is the global bottleneck, so all
    projection work is interleaved as filler and every spare matmul
    column is trimmed (V bias is applied at the drain on DVE instead of
    16 ones-row matmuls; the broadcast ones vector is memset on chip).
  * Host pre-packs X^T and W^T into partition-major [128, 6*N] layouts:
    input DMAs are fully contiguous (max descriptor efficiency), split
    across the sync (X) and scalar (weights) DGE queues.  Mask and
    biases ride in spare columns of the wk param.
  * The (j,qc) drain is software-pipelined across the sweep boundary
    (reciprocal work at iter 0, broadcast matmul + multiply + output
    DMA at iter 1) and the ctx accumulators are copied out of PSUM
    immediately so the banks recycle; the final drain runs its copies
    on the then-idle ACT engine.
  * 1/denom via DVE reciprocal_approx_fast on an SBUF-staged row (the
    custom op drops partition offsets).  GPSIMD compute is avoided:
    touching it downclocks the whole core ~20%.

PSUM (8 banks): scores 2x[128,1024] (4) + ctx 2x[65,512] (2) +
projection/V/broadcast 2x[128,512] (2); the prologue borrows 3 banks
before the attention pools open.
"""

import numpy as np

import concourse.bass as bass
import concourse.tile as tile
from concourse import bacc
from concourse import mybir
from concourse.bass_utils import run_bass_kernel_spmd

F32 = mybir.dt.float32
F16 = mybir.dt.float16
DT_MM = F16
DT_NP = np.float16

BS, SEQ, HID, NH, HD = 4, 2048, 768, 12, 64
NCORES = 8
HPC = 6          # heads per core
FCH = 6          # 128-row chunks of the 768 contraction dim
DSH = HPC * HD   # 384 output features per core
QC = 4           # q chunks of 512
KB = 16          # k blocks of 128
WKX = FCH * DSH  # start of the mask/bias columns in the wk param

FILL = {
    (0, 0): {2: ("k", 0, 1), 6: ("k", 0, 2), 10: ("k", 0, 3),
             14: ("q", 0, 1)},
    (0, 1): {6: ("q", 0, 2), 10: ("k", 1, 0), 13: ("k", 1, 1)},
    (0, 2): {6: ("q", 0, 3), 10: ("k", 1, 2), 13: ("k", 1, 3)},
    (0, 3): {8: ("q", 1, 0)},
    (1, 0): {6: ("q", 1, 1), 10: ("k", 2, 0), 13: ("k", 2, 1)},
    (1, 1): {6: ("q", 1, 2), 10: ("k", 2, 2), 13: ("k", 2, 3)},
    (1, 2): {8: ("q", 1, 3)},
    (1, 3): {8: ("q", 2, 0)},
    (2, 0): {8: ("q", 2, 1)},
    (2, 1): {8: ("q", 2, 2)},
    (2, 2): {8: ("q", 2, 3)},
    (2, 3): {},
}


def _body(tc, xt_d, wq_d, wk_d, wv_d, ot_d):
    nc = tc.nc
    Exp = mybir.ActivationFunctionType.Exp

    with tc.tile_pool(name="persist", bufs=1) as persist:
        # Warm the exp table ASAP (overlaps the input DMAs).
        dummy = persist.tile([1, 1], F32, tag="dummy")
        nc.vector.memset(dummy, 0.0)
        nc.scalar.activation(out=dummy, in_=dummy, func=Exp)

        # Contiguous partition-major input DMAs on two parallel DGE
        # queues: X (split for chunk-level pipelining) on sync, weights
        # on scalar.  GPSIMD's queue is untouched: ANY GpSimd engagement
        # (even a DMA trigger) downclocks the whole core ~20%.
        # X arrives in 4 position-blocks (columns 512c..512c+511): the
        # first scores need only block 0, later blocks feed the V/K JIT.
        # Two parallel DGE queues: X on sync, weights on scalar (the
        # gpsimd queue starts too slowly to help).
        wkp = persist.tile([128, WKX + 28], DT_MM, tag="wkp")
        nc.scalar.dma_start(out=wkp, in_=wk_d[:, :])
        xtp = persist.tile([128, QC, FCH, 512], DT_MM, tag="xtp")
        for c in range(3):
            nc.sync.dma_start(out=xtp[:, c, :, :], in_=xt_d[:, c, :, :])
        wqp = persist.tile([128, FCH, DSH], DT_MM, tag="wqp")
        nc.scalar.dma_start(out=wqp, in_=wq_d[:, :, :])
        wvp = persist.tile([128, FCH, DSH], DT_MM, tag="wvp")
        nc.scalar.dma_start(out=wvp, in_=wv_d[:, :, :])
        # scalar queue idles after the weights while sync still streams
        # X: it takes the last X block (needed latest, at kb=12).
        nc.scalar.dma_start(out=xtp[:, 3, :, :], in_=xt_d[:, 3, :, :])

        # Mask / bias views and casts (wkp cols WKX..): 16 mask, 3 bq,
        # 3 bk, 6 bv (64 rows each).
        mtile = wkp[:, WKX:WKX + KB]
        mtf = persist.tile([128, KB], F32, tag="mtf")
        nc.vector.tensor_copy(out=mtf, in_=mtile)
        qkb = persist.tile([128, 6], F32, tag="qkb")
        nc.vector.tensor_copy(out=qkb, in_=wkp[:, WKX + KB:WKX + KB + 6])
        bvt = persist.tile([128, 6], F32, tag="bvt")
        nc.vector.tensor_copy(out=bvt, in_=wkp[:, WKX + KB + 6:WKX + 28])
        ones64 = persist.tile([1, HD], DT_MM, tag="ones64")
        nc.vector.memset(ones64, 1.0)

        def xq(qc, f):
            # X^T rows f*128.., columns qc*512..(qc+1)*512
            return xtp[:, qc, f, :]

        def xk(kb, f):
            # X^T rows f*128.., columns kb*128..(kb+1)*128
            return xtp[:, kb // 4, f, (kb % 4) * 128:(kb % 4 + 1) * 128]

        qt = [persist.tile([128, SEQ], DT_MM, tag=f"qt{j}", name=f"qt{j}")
              for j in range(3)]
        kt = [persist.tile([128, SEQ], DT_MM, tag=f"kt{j}", name=f"kt{j}")
              for j in range(3)]
        # V with per-head mask column: [k=128, kb, head, 64 V dims + m].
        vt = persist.tile([128, KB, HPC, HD + 1], DT_MM, tag="vt")
        for h in range(HPC):
            nc.vector.tensor_copy(out=vt[:, :, h, HD], in_=mtile)
        ostage = [persist.tile([64, SEQ], F32, tag=f"os{h}", name=f"os{h}")
                  for h in range(HPC)]

        def make_proj(fpool):
            def proj_chunk(kind, j, qc):
                """Q or K projection chunk -> qt/kt[j][:, qc*512:...],
                bias folded into the DVE drain."""
                ps = fpool.tile([128, 512], F32, tag="f", name="fq")
                qs = slice(qc * 512, (qc + 1) * 512)
                wt = wqp if kind == "q" else wkp
                for f in range(FCH):
                    nc.tensor.matmul(
                        ps,
                        lhsT=wt[:, f * DSH + j * 128:f * DSH + (j + 1) * 128]
                        if kind == "k" else wt[:, f, j * 128:(j + 1) * 128],
                        rhs=xq(qc, f),
                        start=(f == 0), stop=(f == FCH - 1))
                dst = (qt if kind == "q" else kt)[j]
                bcol = (0 if kind == "q" else 3) + j
                nc.vector.tensor_scalar_add(out=dst[:, qs], in0=ps,
                                            scalar1=qkb[:, bcol:bcol + 1])

            def v_chunk(kb):
                """V k-block kb -> vt[:, kb, :, 0:64], mask-scaled rows.
                bv is applied at the drain (out = ctx/denom + bv)."""
                ps = fpool.tile([128, DSH], F32, tag="f", name="fv")
                for f in range(FCH):
                    nc.tensor.matmul(ps, lhsT=xk(kb, f),
                                     rhs=wvp[:, f, :],
                                     start=(f == 0), stop=(f == FCH - 1))
                nc.vector.tensor_scalar_mul(
                    out=vt[:, kb, :, 0:HD], in0=ps,
                    scalar1=mtf[:, kb:kb + 1])

            return proj_chunk, v_chunk

        # Prologue (overlaps the input DMA stream) in its own multi-buffer
        # PSUM pool so chunks pipeline at PE speed.
        with tc.tile_pool(name="pre", bufs=3, space="PSUM") as pre:
            proj_chunk, v_chunk = make_proj(pre)
            proj_chunk("k", 0, 0)
            proj_chunk("q", 0, 0)
            v_chunk(0)
            v_chunk(1)

        with tc.tile_pool(name="sp", bufs=2, space="PSUM") as sp, \
             tc.tile_pool(name="cp", bufs=2, space="PSUM") as cp, \
             tc.tile_pool(name="fp", bufs=2, space="PSUM") as fp, \
             tc.tile_pool(name="pp", bufs=8) as pp, \
             tc.tile_pool(name="rdp", bufs=3) as rdp:
            proj_chunk, v_chunk = make_proj(fp)

            def drain_p1(st, on_act=False):
                """First drain stage: copy ctx out of PSUM (frees the
                accumulator banks) and build the f16 reciprocal rows."""
                cpy = nc.scalar.copy if on_act else (
                    lambda out, in_: nc.vector.tensor_copy(out=out, in_=in_))
                st["cs"], st["rd"] = [], []
                for i in range(2):
                    cs = rdp.tile([64, 512], F32, tag=f"cs{i}", name="cs")
                    cpy(out=cs, in_=st["ctx"][i][0:HD, :])
                    dn = rdp.tile([1, 512], F32, tag="dn", name="dn")
                    cpy(out=dn, in_=st["ctx"][i][HD:HD + 1, :])
                    r32 = rdp.tile([1, 512], F32, tag="r32", name="r32")
                    nc.vector.reciprocal_approx_fast(out=r32, in_=dn)
                    rd = rdp.tile([1, 512], DT_MM, tag="r16", name="rd")
                    nc.vector.tensor_copy(out=rd, in_=r32)
                    st["cs"].append(cs)
                    st["rd"].append(rd)

            def drain_p2(st, on_act=False):
                """Second drain stage: broadcast 1/denom (ones matmul),
                multiply, add bv, stream the output DMA."""
                cpy = nc.scalar.copy if on_act else (
                    lambda out, in_: nc.vector.tensor_copy(out=out, in_=in_))
                for i in range(2):
                    h = st["heads"][i]
                    bc = fp.tile([64, 512], F32, tag="f", name="bc")
                    nc.tensor.matmul(bc, lhsT=ones64, rhs=st["rd"][i],
                                     start=True, stop=True)
                    bcs = rdp.tile([64, 512], F32, tag="bcs", name="bcs")
                    cpy(out=bcs, in_=bc)
                    mo = rdp.tile([64, 512], F32, tag="mo", name="mo")
                    nc.vector.tensor_mul(out=mo, in0=st["cs"][i], in1=bcs)
                    nc.vector.tensor_scalar_add(
                        out=ostage[h][:, st["qs"]], in0=mo,
                        scalar1=bvt[0:HD, h:h + 1])
                    nc.sync.dma_start(out=ot_d[h][:, st["qs"]],
                                      in_=ostage[h][:, st["qs"]])

            # Uniform software pipeline over all 192 (j,qc,kb) iterations:
            # at iteration t the PE stream is S(t), C(t-1), filler —
            # including across sweep boundaries.
            prevc = None
            pending = None
            for j in range(3):
                heads = (2 * j, 2 * j + 1)
                for qc in range(QC):
                    qs = slice(qc * 512, (qc + 1) * 512)
                    fill_at = FILL[(j, qc)]
                    ctx = [cp.tile([HD + 1, 512], F32, tag="c", name=f"ctx{i}")
                           for i in range(2)]
                    for kb in range(KB):
                        ks = slice(kb * 128, (kb + 1) * 128)
                        sab = sp.tile([128, 1024], F32, tag="s", name="sab")
                        # Scores gate the ACT stream (the critical engine):
                        # highest priority so ready fillers never preempt.
                        with tc.high_priority():
                            for i in range(2):
                                rows = slice(64 * i, 64 * (i + 1))
                                nc.tensor.matmul(sab[:, 512 * i:512 * (i + 1)],
                                                 lhsT=kt[j][rows, ks],
                                                 rhs=qt[j][rows, qs],
                                                 start=True, stop=True,
                                                 skip_group_check=True)
                        if prevc is not None:
                            pctx, pheads, pkb, pp_ = prevc
                            for i in range(2):
                                nc.tensor.matmul(
                                    pctx[i],
                                    lhsT=vt[:, pkb, pheads[i], :],
                                    rhs=pp_[:, 512 * i:512 * (i + 1)],
                                    start=(pkb == 0), stop=(pkb == KB - 1))
                        if kb == 0 and pending is not None:
                            drain_p1(pending)
                        if kb == 1 and pending is not None:
                            drain_p2(pending)
                            pending = None
                        if j == 0 and qc == 0 and kb < KB - 2:
                            v_chunk(kb + 2)
                        if kb in fill_at:
                            proj_chunk(*fill_at[kb])
                        p = pp.tile([128, 1024], DT_MM, tag="p", name="ptile")
                        nc.scalar.activation(out=p, in_=sab, func=Exp,
                                             scale=0.125)
                        prevc = (ctx, heads, kb, p)
                    pending = {"ctx": ctx, "heads": heads, "qs": qs}

            # Tail: final ctx pair, then the last drain with its copies on
            # the now-idle ACT engine.
            pctx, pheads, pkb, pp_ = prevc
            for i in range(2):
                nc.tensor.matmul(pctx[i], lhsT=vt[:, pkb, pheads[i], :],
                                 rhs=pp_[:, 512 * i:512 * (i + 1)],
                                 start=False, stop=True)
            drain_p1(pending, on_act=True)
            drain_p2(pending, on_act=True)


def build_nc():
    nc = bacc.Bacc("TRN2")
    xt_d = nc.declare_dram_parameter("xtp", [128, QC, FCH, 512], DT_MM, isOutput=False)
    wq_d = nc.declare_dram_parameter("wqp", [128, FCH, DSH], DT_MM, isOutput=False)
    wk_d = nc.declare_dram_parameter("wkp", [128, WKX + 28], DT_MM, isOutput=False)
    wv_d = nc.declare_dram_parameter("wvp", [128, FCH, DSH], DT_MM, isOutput=False)
    ot_d = nc.declare_dram_parameter("OT", [HPC, HD, SEQ], F32, isOutput=True)
    with tile.TileContext(nc) as tc:
        _body(tc, xt_d, wq_d, wk_d, wv_d, ot_d)
    nc.finalize()
    return nc


_NC_CACHE = None


def _get_nc():
    global _NC_CACHE
    if _NC_CACHE is None:
        _NC_CACHE = build_nc()
    return _NC_CACHE


def _pack_pm(m):
    """[768, N] -> partition-major [128, 6*N] (chunk-major free dim)."""
    n = m.shape[1]
    return np.ascontiguousarray(
        m.reshape(FCH, 128, n).transpose(1, 0, 2).reshape(128, FCH * n))


def make_in_maps(hidden_states, attention_mask, Wq, bq, Wk, bk, Wv, bv):
    in_maps = []
    for c in range(NCORES):
        b, g = c // 2, c % 2
        hs = slice(g * DSH, (g + 1) * DSH)
        # [128, qc-block, f-chunk, 512]: xtp[p,c,f,q] = X^T[f*128+p, c*512+q]
        xtp = np.ascontiguousarray(
            hidden_states[b].T.astype(DT_NP)
            .reshape(FCH, 128, QC, 512).transpose(1, 2, 0, 3))
        wqp = _pack_pm(Wq[hs, :].T.astype(DT_NP))
        wvp = _pack_pm(Wv[hs, :].T.astype(DT_NP))

        wkp = np.zeros((128, WKX + 28), DT_NP)
        wkp[:, :WKX] = _pack_pm(Wk[hs, :].T.astype(DT_NP))
        m = (attention_mask[b, 0, 0] > -1).astype(DT_NP)
        wkp[:, WKX:WKX + KB] = m.reshape(KB, 128).T
        for j in range(3):
            wkp[:, WKX + KB + j] = bq[g * DSH + j * 128: g * DSH + (j + 1) * 128]
            wkp[:, WKX + KB + 3 + j] = bk[g * DSH + j * 128: g * DSH + (j + 1) * 128]
        for h in range(HPC):
            wkp[0:HD, WKX + KB + 6 + h] = bv[g * DSH + h * HD: g * DSH + (h + 1) * HD]

        in_maps.append({"xtp": xtp, "wqp": wqp, "wkp": wkp, "wvp": wvp})
    return in_maps


def gather_out(results):
    out = np.empty((BS, SEQ, HID), np.float32)
    for c in range(NCORES):
        b, g = c // 2, c % 2
        ot = results[c]["OT"]  # [6, 64, 2048]
        out[b, :, g * DSH:(g + 1) * DSH] = (
            ot.transpose(2, 0, 1).reshape(SEQ, DSH)
        )
    return out


def kernel(hidden_states, attention_mask, Wq, bq, Wk, bk, Wv, bv):
    nc = _get_nc()
    in_maps = make_in_maps(hidden_states, attention_mask,
                           Wq, bq, Wk, bk, Wv, bv)
    res = run_bass_kernel_spmd(nc, in_maps, core_ids=list(range(NCORES)))
    return gather_out(res.results)


# revision 46
# speedup vs baseline: 1.1999x; 1.0099x over previous
"""BERT self-attention (BS=4, SEQ=2048, HID=768, NH=12) on 8 NeuronCores.

Sharding: core c -> batch b = c//2, head-group g = c%2 (6 heads each).

v8 design (573us baseline -> 337 -> 295 -> 292 -> 285 -> this):
  * Softmax denominator comes free from the ctx matmul: V is stored per
    head as 65 columns (64 V dims + the 0/1 mask column), so ctx PSUM
    row 64 accumulates sum_k m_k * P[k,q].  No denominator matmuls.
  * Scores for the head pair are packed side by side in one
    [128k, 2*512q] PSUM tile -> ONE exp per kb iteration; the two
    64-dim score matmuls run concurrently in PE row halves.
  * The PE (~0.5 ns/column streaming) is the global bottleneck, so all
    projection work is interleaved as filler and every spare matmul
    column is trimmed (V bias is applied at the drain on DVE instead of
    16 ones-row matmuls; the broadcast ones vector is memset on chip).
  * Host pre-packs X^T and W^T into partition-major [128, 6*N] layouts:
    input DMAs are fully contiguous (max descriptor efficiency), split
    across the sync (X) and scalar (weights) DGE queues.  Mask and
    biases ride in spare columns of the wk param.
  * The (j,qc) drain is software-pipelined across the sweep boundary
    (reciprocal work at iter 0, broadcast matmul + multiply + output
    DMA at iter 1) and the ctx accumulators are copied out of PSUM
    immediately so the banks recycle; the final drain runs its copies
    on the then-idle ACT engine.
  * 1/denom via DVE reciprocal_approx_fast on an SBUF-staged row (the
    custom op drops partition offsets).  GPSIMD compute is avoided:
    touching it downclocks the whole core ~20%.

PSUM (8 banks): scores 2x[128,1024] (4) + ctx 2x[65,512] (2) +
projection/V/broadcast 2x[128,512] (2); the prologue borrows 3 banks
before the attention pools open.
"""

import numpy as np

import concourse.bass as bass
import concourse.tile as tile
from concourse import bacc
from concourse import mybir
from concourse.bass_utils import run_bass_kernel_spmd

F32 = mybir.dt.float32
F16 = mybir.dt.float16
DT_MM = F16
DT_NP = np.float16

BS, SEQ, HID, NH, HD = 4, 2048, 768, 12, 64
NCORES = 8
HPC = 6          # heads per core
FCH = 6          # 128-row chunks of the 768 contraction dim
DSH = HPC * HD   # 384 output features per core
QC = 4           # q chunks of 512
KB = 16          # k blocks of 128
WKX = FCH * DSH  # start of the mask/bias columns in the wk param

FILL = {
    (0, 0): {2: ("k", 0, 1), 6: ("k", 0, 2), 10: ("k", 0, 3),
             14: ("q", 0, 1)},
    (0, 1): {6: ("q", 0, 2), 10: ("k", 1, 0), 13: ("k", 1, 1)},
    (0, 2): {6: ("q", 0, 3), 10: ("k", 1, 2), 13: ("k", 1, 3)},
    (0, 3): {8: ("q", 1, 0)},
    (1, 0): {6: ("q", 1, 1), 10: ("k", 2, 0), 13: ("k", 2, 1)},
    (1, 1): {6: ("q", 1, 2), 10: ("k", 2, 2), 13: ("k", 2, 3)},
    (1, 2): {8: ("q", 1, 3)},
    (1, 3): {8: ("q", 2, 0)},
    (2, 0): {8: ("q", 2, 1)},
    (2, 1): {8: ("q", 2, 2)},
    (2, 2): {8: ("q", 2, 3)},
    (2, 3): {},
}


def _body(tc, xt_d, wq_d, wk_d, wv_d, ot_d):
    nc = tc.nc
    Exp = mybir.ActivationFunctionType.Exp

    with tc.tile_pool(name="persist", bufs=1) as persist:
        # Warm the exp table ASAP (overlaps the input DMAs).
        dummy = persist.tile([1, 1], F32, tag="dummy")
        nc.vector.memset(dummy, 0.0)
        nc.scalar.activation(out=dummy, in_=dummy, func=Exp)

        # Contiguous partition-major input DMAs on two parallel DGE
        # queues: X (split for chunk-level pipelining) on sync, weights
        # on scalar.  GPSIMD's queue is untouched: ANY GpSimd engagement
        # (even a DMA trigger) downclocks the whole core ~20%.
        # X arrives in 4 position-blocks (columns 512c..512c+511): the
        # first scores need only block 0, later blocks feed the V/K JIT.
        # Two parallel DGE queues: X on sync, weights on scalar (the
        # gpsimd queue starts too slowly to help).
        wkp = persist.tile([128, WKX + 28], DT_MM, tag="wkp")
        nc.scalar.dma_start(out=wkp, in_=wk_d[:, :])
        xtp = persist.tile([128, QC, FCH, 512], DT_MM, tag="xtp")
        for c in range(3):
            nc.sync.dma_start(out=xtp[:, c, :, :], in_=xt_d[:, c, :, :])
        wqp = persist.tile([128, FCH, DSH], DT_MM, tag="wqp")
        nc.scalar.dma_start(out=wqp, in_=wq_d[:, :, :])
        wvp = persist.tile([128, FCH, DSH], DT_MM, tag="wvp")
        nc.scalar.dma_start(out=wvp, in_=wv_d[:, :, :])
        # scalar queue idles after the weights while sync still streams
        # X: it takes the last X block (needed latest, at kb=12).
        nc.scalar.dma_start(out=xtp[:, 3, :, :], in_=xt_d[:, 3, :, :])

        # Mask / bias views and casts (wkp cols WKX..): 16 mask, 3 bq,
        # 3 bk, 6 bv (64 rows each).
        mtile = wkp[:, WKX:WKX + KB]
        mtf = persist.tile([128, KB], F32, tag="mtf")
        nc.vector.tensor_copy(out=mtf, in_=mtile)
        qkb = persist.tile([128, 6], F32, tag="qkb")
        nc.vector.tensor_copy(out=qkb, in_=wkp[:, WKX + KB:WKX + KB + 6])
        bvt = persist.tile([128, 6], F32, tag="bvt")
        nc.vector.tensor_copy(out=bvt, in_=wkp[:, WKX + KB + 6:WKX + 28])
        ones64 = persist.tile([1, HD], DT_MM, tag="ones64")
        nc.vector.memset(ones64, 1.0)

        def xq(qc, f):
            # X^T rows f*128.., columns qc*512..(qc+1)*512
            return xtp[:, qc, f, :]

        def xk(kb, f):
            # X^T rows f*128.., columns kb*128..(kb+1)*128
            return xtp[:, kb // 4, f, (kb % 4) * 128:(kb % 4 + 1) * 128]

        qt = [persist.tile([128, SEQ], DT_MM, tag=f"qt{j}", name=f"qt{j}")
              for j in range(3)]
        kt = [persist.tile([128, SEQ], DT_MM, tag=f"kt{j}", name=f"kt{j}")
              for j in range(3)]
        # V with per-head mask column: [k=128, kb, head, 64 V dims + m].
        vt = persist.tile([128, KB, HPC, HD + 1], DT_MM, tag="vt")
        for h in range(HPC):
            nc.vector.tensor_copy(out=vt[:, :, h, HD], in_=mtile)
        ostage = [persist.tile([64, SEQ], F32, tag=f"os{h}", name=f"os{h}")
                  for h in range(HPC)]

        def make_proj(fpool):
            def proj_chunk(kind, j, qc):
                """Q or K projection chunk -> qt/kt[j][:, qc*512:...],
                bias folded into the DVE drain."""
                ps = fpool.tile([128, 512], F32, tag="f", name="fq")
                qs = slice(qc * 512, (qc + 1) * 512)
                wt = wqp if kind == "q" else wkp
                for f in range(FCH):
                    nc.tensor.matmul(
                        ps,
                        lhsT=wt[:, f * DSH + j * 128:f * DSH + (j + 1) * 128]
                        if kind == "k" else wt[:, f, j * 128:(j + 1) * 128],
                        rhs=xq(qc, f),
                        start=(f == 0), stop=(f == FCH - 1))
                dst = (qt if kind == "q" else kt)[j]
                bcol = (0 if kind == "q" else 3) + j
                nc.vector.tensor_scalar_add(out=dst[:, qs], in0=ps,
                                            scalar1=qkb[:, bcol:bcol + 1])

            def v_chunk(kb):
                """V k-block kb -> vt[:, kb, :, 0:64], mask-scaled rows.
                bv is applied at the drain (out = ctx/denom + bv)."""
                ps = fpool.tile([128, DSH], F32, tag="f", name="fv")
                for f in range(FCH):
                    nc.tensor.matmul(ps, lhsT=xk(kb, f),
                                     rhs=wvp[:, f, :],
                                     start=(f == 0), stop=(f == FCH - 1))
                nc.vector.tensor_scalar_mul(
                    out=vt[:, kb, :, 0:HD], in0=ps,
                    scalar1=mtf[:, kb:kb + 1])

            return proj_chunk, v_chunk

        # Prologue (overlaps the input DMA stream) in its own multi-buffer
        # PSUM pool so chunks pipeline at PE speed.
        # Keep the prologue pool minimal: the attention pools can only
        # open once every prologue PSUM tile has drained, so V0/V1 (gated
        # on the late wv DMA) run as sweep-0 fillers instead.
        with tc.tile_pool(name="pre", bufs=2, space="PSUM") as pre:
            proj_chunk, v_chunk = make_proj(pre)
            proj_chunk("k", 0, 0)
            proj_chunk("q", 0, 0)

        with tc.tile_pool(name="sp", bufs=2, space="PSUM") as sp, \
             tc.tile_pool(name="cp", bufs=2, space="PSUM") as cp, \
             tc.tile_pool(name="fp", bufs=2, space="PSUM") as fp, \
             tc.tile_pool(name="pp", bufs=8) as pp, \
             tc.tile_pool(name="rdp", bufs=3) as rdp:
            proj_chunk, v_chunk = make_proj(fp)

            def drain_p1(st, on_act=False):
                """First drain stage: copy ctx out of PSUM (frees the
                accumulator banks) and build the f16 reciprocal rows."""
                cpy = nc.scalar.copy if on_act else (
                    lambda out, in_: nc.vector.tensor_copy(out=out, in_=in_))
                st["cs"], st["rd"] = [], []
                for i in range(2):
                    cs = rdp.tile([64, 512], F32, tag=f"cs{i}", name="cs")
                    cpy(out=cs, in_=st["ctx"][i][0:HD, :])
                    dn = rdp.tile([1, 512], F32, tag="dn", name="dn")
                    cpy(out=dn, in_=st["ctx"][i][HD:HD + 1, :])
                    r32 = rdp.tile([1, 512], F32, tag="r32", name="r32")
                    nc.vector.reciprocal_approx_fast(out=r32, in_=dn)
                    rd = rdp.tile([1, 512], DT_MM, tag="r16", name="rd")
                    nc.vector.tensor_copy(out=rd, in_=r32)
                    st["cs"].append(cs)
                    st["rd"].append(rd)

            def drain_p2(st, on_act=False):
                """Second drain stage: broadcast 1/denom (ones matmul),
                multiply, add bv, stream the output DMA."""
                cpy = nc.scalar.copy if on_act else (
                    lambda out, in_: nc.vector.tensor_copy(out=out, in_=in_))
                for i in range(2):
                    h = st["heads"][i]
                    bc = fp.tile([64, 512], F32, tag="f", name="bc")
                    nc.tensor.matmul(bc, lhsT=ones64, rhs=st["rd"][i],
                                     start=True, stop=True)
                    bcs = rdp.tile([64, 512], F32, tag="bcs", name="bcs")
                    cpy(out=bcs, in_=bc)
                    mo = rdp.tile([64, 512], F32, tag="mo", name="mo")
                    nc.vector.tensor_mul(out=mo, in0=st["cs"][i], in1=bcs)
                    nc.vector.tensor_scalar_add(
                        out=ostage[h][:, st["qs"]], in0=mo,
                        scalar1=bvt[0:HD, h:h + 1])
                    nc.sync.dma_start(out=ot_d[h][:, st["qs"]],
                                      in_=ostage[h][:, st["qs"]])

            # Uniform software pipeline over all 192 (j,qc,kb) iterations:
            # at iteration t the PE stream is S(t), C(t-1), filler —
            # including across sweep boundaries.
            prevc = None
            pending = None
            for j in range(3):
                heads = (2 * j, 2 * j + 1)
                for qc in range(QC):
                    qs = slice(qc * 512, (qc + 1) * 512)
                    fill_at = FILL[(j, qc)]
                    ctx = [cp.tile([HD + 1, 512], F32, tag="c", name=f"ctx{i}")
                           for i in range(2)]
                    for kb in range(KB):
                        ks = slice(kb * 128, (kb + 1) * 128)
                        sab = sp.tile([128, 1024], F32, tag="s", name="sab")
                        # Scores gate the ACT stream (the critical engine):
                        # highest priority so ready fillers never preempt.
                        with tc.high_priority():
                            for i in range(2):
                                rows = slice(64 * i, 64 * (i + 1))
                                nc.tensor.matmul(sab[:, 512 * i:512 * (i + 1)],
                                                 lhsT=kt[j][rows, ks],
                                                 rhs=qt[j][rows, qs],
                                                 start=True, stop=True,
                                                 skip_group_check=True)
                        if prevc is not None:
                            pctx, pheads, pkb, pp_ = prevc
                            for i in range(2):
                                nc.tensor.matmul(
                                    pctx[i],
                                    lhsT=vt[:, pkb, pheads[i], :],
                                    rhs=pp_[:, 512 * i:512 * (i + 1)],
                                    start=(pkb == 0), stop=(pkb == KB - 1))
                        if kb == 0 and pending is not None:
                            drain_p1(pending)
                        if kb == 1 and pending is not None:
                            drain_p2(pending)
                            pending = None
                        if j == 0 and qc == 0:
                            if kb == 0:
                                v_chunk(0)
                                v_chunk(1)
                            if kb < KB - 2:
                                v_chunk(kb + 2)
                        if kb in fill_at:
                            proj_chunk(*fill_at[kb])
                        p = pp.tile([128, 1024], DT_MM, tag="p", name="ptile")
                        nc.scalar.activation(out=p, in_=sab, func=Exp,
                                             scale=0.125)
                        prevc = (ctx, heads, kb, p)
                    pending = {"ctx": ctx, "heads": heads, "qs": qs}

            # Tail: final ctx pair, then the last drain with its copies on
            # the now-idle ACT engine.
            pctx, pheads, pkb, pp_ = prevc
            for i in range(2):
                nc.tensor.matmul(pctx[i], lhsT=vt[:, pkb, pheads[i], :],
                                 rhs=pp_[:, 512 * i:512 * (i + 1)],
                                 start=False, stop=True)
            drain_p1(pending, on_act=True)
            drain_p2(pending, on_act=True)


def build_nc():
    nc = bacc.Bacc("TRN2")
    xt_d = nc.declare_dram_parameter("xtp", [128, QC, FCH, 512], DT_MM, isOutput=False)
    wq_d = nc.declare_dram_parameter("wqp", [128, FCH, DSH], DT_MM, isOutput=False)
    wk_d = nc.declare_dram_parameter("wkp", [128, WKX + 28], DT_MM, isOutput=False)
    wv_d = nc.declare_dram_parameter("wvp", [128, FCH, DSH], DT_MM, isOutput=False)
    ot_d = nc.declare_dram_parameter("OT", [HPC, HD, SEQ], F32, isOutput=True)
    with tile.TileContext(nc) as tc:
        _body(tc, xt_d, wq_d, wk_d, wv_d, ot_d)
    nc.finalize()
    return nc


_NC_CACHE = None


def _get_nc():
    global _NC_CACHE
    if _NC_CACHE is None:
        _NC_CACHE = build_nc()
    return _NC_CACHE


def _pack_pm(m):
    """[768, N] -> partition-major [128, 6*N] (chunk-major free dim)."""
    n = m.shape[1]
    return np.ascontiguousarray(
        m.reshape(FCH, 128, n).transpose(1, 0, 2).reshape(128, FCH * n))


def make_in_maps(hidden_states, attention_mask, Wq, bq, Wk, bk, Wv, bv):
    in_maps = []
    for c in range(NCORES):
        b, g = c // 2, c % 2
        hs = slice(g * DSH, (g + 1) * DSH)
        # [128, qc-block, f-chunk, 512]: xtp[p,c,f,q] = X^T[f*128+p, c*512+q]
        xtp = np.ascontiguousarray(
            hidden_states[b].T.astype(DT_NP)
            .reshape(FCH, 128, QC, 512).transpose(1, 2, 0, 3))
        wqp = _pack_pm(Wq[hs, :].T.astype(DT_NP))
        wvp = _pack_pm(Wv[hs, :].T.astype(DT_NP))

        wkp = np.zeros((128, WKX + 28), DT_NP)
        wkp[:, :WKX] = _pack_pm(Wk[hs, :].T.astype(DT_NP))
        m = (attention_mask[b, 0, 0] > -1).astype(DT_NP)
        wkp[:, WKX:WKX + KB] = m.reshape(KB, 128).T
        for j in range(3):
            wkp[:, WKX + KB + j] = bq[g * DSH + j * 128: g * DSH + (j + 1) * 128]
            wkp[:, WKX + KB + 3 + j] = bk[g * DSH + j * 128: g * DSH + (j + 1) * 128]
        for h in range(HPC):
            wkp[0:HD, WKX + KB + 6 + h] = bv[g * DSH + h * HD: g * DSH + (h + 1) * HD]

        in_maps.append({"xtp": xtp, "wqp": wqp, "wkp": wkp, "wvp": wvp})
    return in_maps


def gather_out(results):
    out = np.empty((BS, SEQ, HID), np.float32)
    for c in range(NCORES):
        b, g = c // 2, c % 2
        ot = results[c]["OT"]  # [6, 64, 2048]
        out[b, :, g * DSH:(g + 1) * DSH] = (
            ot.transpose(2, 0, 1).reshape(SEQ, DSH)
        )
    return out


def kernel(hidden_states, attention_mask, Wq, bq, Wk, bk, Wv, bv):
    nc = _get_nc()
    in_maps = make_in_maps(hidden_states, attention_mask,
                           Wq, bq, Wk, bk, Wv, bv)
    res = run_bass_kernel_spmd(nc, in_maps, core_ids=list(range(NCORES)))
    return gather_out(res.results)
